# revision 15
# baseline (speedup 1.0000x reference)
"""AttentionPool3D kernel for 8 Trainium2 NeuronCores — xbar edition.

Math (per batch b):
  qk      = queries @ Wk                      [Q, C]
  scores  = (qk @ xf) * C**-0.5               [Q, S]   (bk shifts cancel in softmax)
  e       = exp(scores)                        (scores ~ N(0,1): no max needed)
  l       = sum_s e                           [Q]
  t       = sum_s e[q,s] * xf[c,s]            [Q, C]
  attended= (t / l) @ Wv.T + bv               [Q, C]   (bv exact: sum attn = 1)
  out     = attended.flatten() @ Wo.T + bo    [OUT]

Sharding: 8 cores = 4 batches x 2 spatial halves (flash-style partial softmax,
combined on host along with the tiny projections, ~0.005% of total FLOPs).

Device kernel per core (all fp16 data, f32 accumulate), per group of 3 tiles
of T=2048 spatial positions (m = tile-in-group, using the PE's allowed PSUM
output partition bases {0, 32, 64}):
  - scores: sc[32m+q, 512-slice] = qk32_cb.T @ x_cb  (qk zero-padded to 32
    rows; x streams as matmul rhs so it never needs a PE weight load)
  - one exp per slice on ScalarE over all 96 partitions: psum f32 ->
    en96 fp16; accum_out gives the l-partials for free
  - xT via DMA-crossbar transpose (a ucode instruction that occupies the
    issuing HWDGE engine, so xT transposes are split sync/scalar):
    [128, 2, 2048] -> [128s, 32blk, 128c]
  - e-xbar per group: en96 [96, 2048] -> eT [128, 16, 96]
  - t-matmuls: t_ps[4, 256] += eT[:, sch, 32m+q].T @ [xT_cb0 | xT_cb1]
Host: t = out rows, l = sum of accum partials at rows 32m+q.
"""

import os
import sys

import numpy as np

for _p in ("/opt/trn_rl_repo", "/root/.axon_site/_ro/trn_rl_repo"):
    if os.path.isdir(_p) and _p not in sys.path:
        sys.path.append(_p)

import concourse.bass as bass
import concourse.tile as tile
from concourse import bacc, bass_utils, mybir
from concourse.bass import ts
from concourse.bass_utils import run_bass_kernel_spmd

F16 = mybir.dt.float16
F32 = mybir.dt.float32

B, C, D, H, W = 4, 256, 32, 48, 48
S = D * H * W            # 73728
Q, OUT = 4, 512
NCORES = 8
SHALF = S // 2           # 36864 per core
SCALE = C ** -0.5        # 1/16, folded into exp's affine
QP = 32                  # padded query rows (PE psum bases 0/32/64)
GT = 3                   # tiles per group

DEFAULT_CFG = dict(
    tile_t=2048,       # spatial tile size (one DMA / one xT xbar)
    slice_w=512,       # score-matmul psum slice width (1 psum bank)
    sc_bufs=4,
    bufs_x=6,
    bufs_xts=7,
    scalar_xT=(),      # xT xbars issued from scalar (rest sync);
                       # concurrent xbars on two engines corrupt data
    exbar_eng="sync",  # e-xbar engine
    dma="gpsimd",      # x-stream DMA engine (SWDGE keeps the sync queue
                       # free for the xbar ucode stream)
)


def _build_program(reps=1, **over):
    cfg = dict(DEFAULT_CFG, **over)
    T = cfg["tile_t"]
    NT = SHALF // T
    NCH = T // 128           # chunks of 128 spatial per tile
    SW = cfg["slice_w"]
    NSL = T // SW            # score slices per tile
    NG = NT // GT            # groups
    NL = NG * NSL            # accum_out columns
    assert NT % GT == 0
    scalar_xT = set(cfg["scalar_xT"])

    nc = bacc.Bacc("TRN2", target_bir_lowering=False, debug=False,
                   num_devices=NCORES)
    xs = nc.dram_tensor("xs", [128, 2, SHALF], F16, kind="ExternalInput").ap()
    qkT = nc.dram_tensor("qkT", [128, 2, QP], F16, kind="ExternalInput").ap()
    out_tl = nc.dram_tensor("out_tl", [Q, C], F32, kind="ExternalOutput").ap()
    l_out = nc.dram_tensor("l_out", [GT * QP, NL], F32,
                           kind="ExternalOutput").ap()

    with tile.TileContext(nc) as tc:
        with (
            tc.tile_pool(name="consts", bufs=1) as consts,
            tc.tile_pool(name="xin", bufs=cfg["bufs_x"]) as xin_pool,
            tc.tile_pool(name="xts", bufs=cfg["bufs_xts"]) as xts_pool,
            tc.tile_pool(name="en", bufs=2) as en_pool,
            tc.tile_pool(name="et", bufs=2) as et_pool,
            tc.tile_pool(name="osb", bufs=1) as out_pool,
            tc.tile_pool(name="scps", bufs=cfg["sc_bufs"], space="PSUM") as sc_pool,
            tc.tile_pool(name="accps", bufs=1, space="PSUM") as acc_pool,
        ):
            qk_sb = consts.tile([128, 2, QP], F16)
            nc.sync.dma_start(qk_sb[:], qkT[:])
            l_sb = consts.tile([GT * QP, NL], F32)

            t_ps = acc_pool.tile([Q, C], F32)

            def emit_front(gi, g, m):
                """DMA + xT-xbar + score matmuls for tile m of group g."""
                it = g * GT + m
                xt = xin_pool.tile([128, 2, T], F16, name="xt", tag="xt")
                getattr(nc, cfg["dma"]).dma_start(xt[:], xs[:, :, ts(it, T)])
                # xT via crossbar: out[p, blk, j] = xt2d[j, blk*128+p]
                xt_sb = xts_pool.tile([128, 2 * NCH, 128], F16, name="xts",
                                      tag="xts")
                eng = nc.scalar if it in scalar_xT else nc.sync
                eng.dma_start_transpose(
                    xt_sb[:], xt[:].rearrange("p a b -> p (a b)"))
                for sl in range(NSL):
                    sc = cur["sc"][sl]
                    for cb in range(2):
                        nc.tensor.matmul(
                            sc[QP * m:QP * (m + 1), :],
                            lhsT=qk_sb[:, cb, :],
                            rhs=xt[:, cb, ts(sl, SW)],
                            start=(cb == 0), stop=(cb == 1),
                        )
                cur["xts"].append(xt_sb)

            def emit_back(prev):
                """e-xbar + t-matmuls for a finished group."""
                g = prev["g"]
                # eT: out[p, blk, j] = en96[j, blk*128+p]
                et = et_pool.tile([128, NCH, GT * QP], F16, name="et",
                                  tag="et")
                getattr(nc, cfg["exbar_eng"]).dma_start_transpose(
                    et[:], prev["en"][:])
                for m in range(GT):
                    gidx = prev["gidx"] * GT + m
                    xt_v = prev["xts"][m][:].rearrange(
                        "p (a b) c -> p b a c", a=2)
                    for sch in range(NCH):
                        # rhs: [128, 2, 128] = [xT_cb0 | xT_cb1]
                        nc.tensor.matmul(
                            t_ps[:],
                            lhsT=et[:, sch, QP * m:QP * m + Q],
                            rhs=xt_v[:, sch, :, :],
                            start=(gidx == 0 and sch == 0),
                            stop=(gidx == reps * NT - 1 and sch == NCH - 1),
                        )

            groups = [(rep, g) for rep in range(reps) for g in range(NG)]
            prev = None
            for gidx in range(len(groups) + 1):
                cur = None
                if gidx < len(groups):
                    rep, g = groups[gidx]
                    cur = dict(
                        g=g, gidx=gidx, xts=[],
                        en=en_pool.tile([GT * QP, T], F16, name="en96",
                                        tag="en96"),
                        sc=[sc_pool.tile([GT * QP, SW], F32, name="sc",
                                         tag="sc") for sl in range(NSL)],
                    )
                    emit_front(gidx, g, 0)
                if prev is not None:
                    emit_back(prev)
                if cur is not None:
                    for m in range(1, GT):
                        emit_front(gidx, g, m)
                    for sl in range(NSL):
                        li = g * NSL + sl
                        nc.scalar.activation(
                            cur["en"][:, ts(sl, SW)], cur["sc"][sl][:],
                            mybir.ActivationFunctionType.Exp, scale=SCALE,
                            accum_out=l_sb[:, li:li + 1])
                prev = cur

            out_sb = out_pool.tile([Q, C], F32)
            nc.vector.tensor_copy(out_sb[:], t_ps[:])
            nc.sync.dma_start(out_tl[:], out_sb[:])
            nc.sync.dma_start(l_out[:], l_sb[:])

    nc.compile()
    return nc


_NC_CACHE = {}


def _get_program(reps=1, **over):
    key = (reps, tuple(sorted(over.items())))
    if key not in _NC_CACHE:
        _NC_CACHE[key] = _build_program(reps, **over)
    return _NC_CACHE[key]


def _make_in_maps(x, queries, Wk):
    xf = np.ascontiguousarray(x.reshape(B, C, S))
    qk = (queries.astype(np.float64) @ Wk.astype(np.float64)).astype(np.float16)
    qk32 = np.zeros((QP, C), np.float16)
    qk32[0:Q] = qk
    # qkT[p, blk, j] = qk32[j, blk*128 + p]
    qkT = np.ascontiguousarray(qk32.T.reshape(2, 128, QP).transpose(1, 0, 2))
    in_maps = []
    for core in range(NCORES):
        b, h = divmod(core, 2)
        shard = xf[b, :, h * SHALF:(h + 1) * SHALF].astype(np.float16)
        # xs[p, blk, s] = xf[b, blk*128 + p, h*SHALF + s]
        xs = np.ascontiguousarray(shard.reshape(2, 128, SHALF).transpose(1, 0, 2))
        in_maps.append({"xs": xs, "qkT": qkT})
    return in_maps


def run_device(in_maps, trace=False, reps=1, **over):
    nc = _get_program(reps, **over)
    return run_bass_kernel_spmd(nc, in_maps, list(range(NCORES)),
                                trace=trace)


def _combine(results, Wv, bv, Wo, bo):
    Wv64 = Wv.astype(np.float64)
    Wo64 = Wo.astype(np.float64)
    out = np.empty((B, OUT), np.float32)
    for b in range(B):
        t = np.zeros((Q, C), np.float64)
        l = np.zeros(Q, np.float64)
        for r in (results[2 * b], results[2 * b + 1]):
            t += r["out_tl"].astype(np.float64)
            lo = r["l_out"].astype(np.float64)        # [GT*QP, NL]
            for q in range(Q):
                l[q] += sum(lo[QP * m + q, :].sum() for m in range(GT))
        attended = (t / l[:, None]) @ Wv64.T + bv.astype(np.float64)
        flat = attended.reshape(-1)          # [Q*C]
        out[b] = (flat @ Wo64.T + bo.astype(np.float64)).astype(np.float32)
    return out


def kernel(x, queries, Wk, bk, Wv, bv, Wo, bo):
    x = np.asarray(x, np.float32)
    queries = np.asarray(queries, np.float32)
    Wk = np.asarray(Wk, np.float32)
    Wv = np.asarray(Wv, np.float32)
    bv = np.asarray(bv, np.float32)
    Wo = np.asarray(Wo, np.float32)
    bo = np.asarray(bo, np.float32)
    # bk shifts every score of a (b, q) row by the same constant, which
    # cancels exactly in softmax; it does not affect the output.
    in_maps = _make_in_maps(x, queries, Wk)
    results = run_device(in_maps).results
    return _combine(results, Wv, bv, Wo, bo)


# revision 16
# speedup vs baseline: 1.0136x; 1.0136x over previous
"""AttentionPool3D kernel for 8 Trainium2 NeuronCores — xbar edition.

Math (per batch b):
  qk      = queries @ Wk                      [Q, C]
  scores  = (qk @ xf) * C**-0.5               [Q, S]   (bk shifts cancel in softmax)
  e       = exp(scores)                        (scores ~ N(0,1): no max needed)
  l       = sum_s e                           [Q]
  t       = sum_s e[q,s] * xf[c,s]            [Q, C]
  attended= (t / l) @ Wv.T + bv               [Q, C]   (bv exact: sum attn = 1)
  out     = attended.flatten() @ Wo.T + bo    [OUT]

Sharding: 8 cores = 4 batches x 2 spatial halves (flash-style partial softmax,
combined on host along with the tiny projections, ~0.005% of total FLOPs).

Device kernel per core (all fp16 data, f32 accumulate), per group of 3 tiles
of T=2048 spatial positions (m = tile-in-group, using the PE's allowed PSUM
output partition bases {0, 32, 64}):
  - scores: sc[32m+q, 512-slice] = qk32_cb.T @ x_cb  (qk zero-padded to 32
    rows; x streams as matmul rhs so it never needs a PE weight load)
  - one exp per slice on ScalarE over all 96 partitions: psum f32 ->
    en96 fp16; accum_out gives the l-partials for free
  - xT via DMA-crossbar transpose (a ucode instruction that occupies the
    issuing HWDGE engine, so xT transposes are split sync/scalar):
    [128, 2, 2048] -> [128s, 32blk, 128c]
  - e-xbar per group: en96 [96, 2048] -> eT [128, 16, 96]
  - t-matmuls: t_ps[4, 256] += eT[:, sch, 32m+q].T @ [xT_cb0 | xT_cb1]
Host: t = out rows, l = sum of accum partials at rows 32m+q.
"""

import os
import sys

import numpy as np

for _p in ("/opt/trn_rl_repo", "/root/.axon_site/_ro/trn_rl_repo"):
    if os.path.isdir(_p) and _p not in sys.path:
        sys.path.append(_p)

import concourse.bass as bass
import concourse.tile as tile
from concourse import bacc, bass_utils, mybir
from concourse.bass import ts
from concourse.bass_utils import run_bass_kernel_spmd

F16 = mybir.dt.float16
F32 = mybir.dt.float32

B, C, D, H, W = 4, 256, 32, 48, 48
S = D * H * W            # 73728
Q, OUT = 4, 512
NCORES = 8
SHALF = S // 2           # 36864 per core
SCALE = C ** -0.5        # 1/16, folded into exp's affine
QP = 32                  # padded query rows (PE psum bases 0/32/64)
GT = 3                   # tiles per group

DEFAULT_CFG = dict(
    tile_t=2048,       # spatial tile size (one DMA / one xT xbar)
    slice_w=512,       # score-matmul psum slice width (1 psum bank)
    sc_bufs=4,
    bufs_x=6,
    bufs_xts=7,
    scalar_xT=(),      # xT xbars issued from scalar (rest sync);
                       # concurrent xbars on two engines corrupt data
    exbar_eng="sync",  # e-xbar engine
    dma="scalar",      # x-stream DMA engine: plain HWDGE ring push (cheap),
                       # kept off sync so the xbar ucode stream never waits
)


def _build_program(reps=1, **over):
    cfg = dict(DEFAULT_CFG, **over)
    T = cfg["tile_t"]
    NT = SHALF // T
    NCH = T // 128           # chunks of 128 spatial per tile
    SW = cfg["slice_w"]
    NSL = T // SW            # score slices per tile
    NG = NT // GT            # groups
    NL = NG * NSL            # accum_out columns
    assert NT % GT == 0
    scalar_xT = set(cfg["scalar_xT"])

    nc = bacc.Bacc("TRN2", target_bir_lowering=False, debug=False,
                   num_devices=NCORES)
    xs = nc.dram_tensor("xs", [128, 2, SHALF], F16, kind="ExternalInput").ap()
    qkT = nc.dram_tensor("qkT", [128, 2, QP], F16, kind="ExternalInput").ap()
    out_tl = nc.dram_tensor("out_tl", [Q, C], F32, kind="ExternalOutput").ap()
    l_out = nc.dram_tensor("l_out", [GT * QP, NL], F32,
                           kind="ExternalOutput").ap()

    with tile.TileContext(nc) as tc:
        with (
            tc.tile_pool(name="consts", bufs=1) as consts,
            tc.tile_pool(name="xin", bufs=cfg["bufs_x"]) as xin_pool,
            tc.tile_pool(name="xts", bufs=cfg["bufs_xts"]) as xts_pool,
            tc.tile_pool(name="en", bufs=2) as en_pool,
            tc.tile_pool(name="et", bufs=2) as et_pool,
            tc.tile_pool(name="osb", bufs=1) as out_pool,
            tc.tile_pool(name="scps", bufs=cfg["sc_bufs"], space="PSUM") as sc_pool,
            tc.tile_pool(name="accps", bufs=1, space="PSUM") as acc_pool,
        ):
            qk_sb = consts.tile([128, 2, QP], F16)
            nc.sync.dma_start(qk_sb[:], qkT[:])
            l_sb = consts.tile([GT * QP, NL], F32)

            t_ps = acc_pool.tile([Q, C], F32)

            def emit_front(gi, g, m):
                """DMA + xT-xbar + score matmuls for tile m of group g."""
                it = g * GT + m
                xt = xin_pool.tile([128, 2, T], F16, name="xt", tag="xt")
                getattr(nc, cfg["dma"]).dma_start(xt[:], xs[:, :, ts(it, T)])
                # xT via crossbar: out[p, blk, j] = xt2d[j, blk*128+p]
                xt_sb = xts_pool.tile([128, 2 * NCH, 128], F16, name="xts",
                                      tag="xts")
                eng = nc.scalar if it in scalar_xT else nc.sync
                eng.dma_start_transpose(
                    xt_sb[:], xt[:].rearrange("p a b -> p (a b)"))
                for sl in range(NSL):
                    sc = cur["sc"][sl]
                    for cb in range(2):
                        nc.tensor.matmul(
                            sc[QP * m:QP * (m + 1), :],
                            lhsT=qk_sb[:, cb, :],
                            rhs=xt[:, cb, ts(sl, SW)],
                            start=(cb == 0), stop=(cb == 1),
                        )
                cur["xts"].append(xt_sb)

            def emit_back(prev):
                """e-xbar + t-matmuls for a finished group."""
                g = prev["g"]
                # eT: out[p, blk, j] = en96[j, blk*128+p]
                et = et_pool.tile([128, NCH, GT * QP], F16, name="et",
                                  tag="et")
                getattr(nc, cfg["exbar_eng"]).dma_start_transpose(
                    et[:], prev["en"][:])
                for m in range(GT):
                    gidx = prev["gidx"] * GT + m
                    xt_v = prev["xts"][m][:].rearrange(
                        "p (a b) c -> p b a c", a=2)
                    for sch in range(NCH):
                        # rhs: [128, 2, 128] = [xT_cb0 | xT_cb1]
                        nc.tensor.matmul(
                            t_ps[:],
                            lhsT=et[:, sch, QP * m:QP * m + Q],
                            rhs=xt_v[:, sch, :, :],
                            start=(gidx == 0 and sch == 0),
                            stop=(gidx == reps * NT - 1 and sch == NCH - 1),
                        )

            groups = [(rep, g) for rep in range(reps) for g in range(NG)]
            prev = None
            for gidx in range(len(groups) + 1):
                cur = None
                if gidx < len(groups):
                    rep, g = groups[gidx]
                    cur = dict(
                        g=g, gidx=gidx, xts=[],
                        en=en_pool.tile([GT * QP, T], F16, name="en96",
                                        tag="en96"),
                        sc=[sc_pool.tile([GT * QP, SW], F32, name="sc",
                                         tag="sc") for sl in range(NSL)],
                    )
                    emit_front(gidx, g, 0)
                if prev is not None:
                    emit_back(prev)
                if cur is not None:
                    for m in range(1, GT):
                        emit_front(gidx, g, m)
                    for sl in range(NSL):
                        li = g * NSL + sl
                        nc.scalar.activation(
                            cur["en"][:, ts(sl, SW)], cur["sc"][sl][:],
                            mybir.ActivationFunctionType.Exp, scale=SCALE,
                            accum_out=l_sb[:, li:li + 1])
                prev = cur

            out_sb = out_pool.tile([Q, C], F32)
            nc.vector.tensor_copy(out_sb[:], t_ps[:])
            nc.sync.dma_start(out_tl[:], out_sb[:])
            nc.sync.dma_start(l_out[:], l_sb[:])

    nc.compile()
    return nc


_NC_CACHE = {}


def _get_program(reps=1, **over):
    key = (reps, tuple(sorted(over.items())))
    if key not in _NC_CACHE:
        _NC_CACHE[key] = _build_program(reps, **over)
    return _NC_CACHE[key]


def _make_in_maps(x, queries, Wk):
    xf = np.ascontiguousarray(x.reshape(B, C, S))
    qk = (queries.astype(np.float64) @ Wk.astype(np.float64)).astype(np.float16)
    qk32 = np.zeros((QP, C), np.float16)
    qk32[0:Q] = qk
    # qkT[p, blk, j] = qk32[j, blk*128 + p]
    qkT = np.ascontiguousarray(qk32.T.reshape(2, 128, QP).transpose(1, 0, 2))
    in_maps = []
    for core in range(NCORES):
        b, h = divmod(core, 2)
        shard = xf[b, :, h * SHALF:(h + 1) * SHALF].astype(np.float16)
        # xs[p, blk, s] = xf[b, blk*128 + p, h*SHALF + s]
        xs = np.ascontiguousarray(shard.reshape(2, 128, SHALF).transpose(1, 0, 2))
        in_maps.append({"xs": xs, "qkT": qkT})
    return in_maps


def run_device(in_maps, trace=False, reps=1, **over):
    nc = _get_program(reps, **over)
    return run_bass_kernel_spmd(nc, in_maps, list(range(NCORES)),
                                trace=trace)


def _combine(results, Wv, bv, Wo, bo):
    Wv64 = Wv.astype(np.float64)
    Wo64 = Wo.astype(np.float64)
    out = np.empty((B, OUT), np.float32)
    for b in range(B):
        t = np.zeros((Q, C), np.float64)
        l = np.zeros(Q, np.float64)
        for r in (results[2 * b], results[2 * b + 1]):
            t += r["out_tl"].astype(np.float64)
            lo = r["l_out"].astype(np.float64)        # [GT*QP, NL]
            for q in range(Q):
                l[q] += sum(lo[QP * m + q, :].sum() for m in range(GT))
        attended = (t / l[:, None]) @ Wv64.T + bv.astype(np.float64)
        flat = attended.reshape(-1)          # [Q*C]
        out[b] = (flat @ Wo64.T + bo.astype(np.float64)).astype(np.float32)
    return out


def kernel(x, queries, Wk, bk, Wv, bv, Wo, bo):
    x = np.asarray(x, np.float32)
    queries = np.asarray(queries, np.float32)
    Wk = np.asarray(Wk, np.float32)
    Wv = np.asarray(Wv, np.float32)
    bv = np.asarray(bv, np.float32)
    Wo = np.asarray(Wo, np.float32)
    bo = np.asarray(bo, np.float32)
    # bk shifts every score of a (b, q) row by the same constant, which
    # cancels exactly in softmax; it does not affect the output.
    in_maps = _make_in_maps(x, queries, Wk)
    results = run_device(in_maps).results
    return _combine(results, Wv, bv, Wo, bo)
